# revision 28
# baseline (speedup 1.0000x reference)
# Trainium2 Bass kernel for nn_CVXPolicy_MultiQuadcopter.
#
# Math (per sample):
#   x  = concat([t, z])                      (3073,)
#   h1 = tanh(x @ W1 + b1)                   (100,)
#   h2 = tanh(h1 @ W2 + b2)                  (100,)
#   p  = h2 @ W3 + b3                        (3072,)
#   c  = S(p)   (per-agent sparse linear map)   (1024,)
#   s  = ||c||^2 ; w = W(256*s) ; k = sqrt(256*w/s)
#   u* = -k * c
#
# Key transformations vs a naive port:
#   - c = S(p) is linear, so S is folded into W3/b3 on the host.
#   - b3 is folded into mm3 via a ones-row (tanh(0+20)==1.0 exactly).
#   - k = 256*exp(-w/2), so the Lambert-W solve needs no sqrt/ln: a
#     damped Newton iteration from a quadratic seed (exp/tanh live in
#     one activation table set -> no mid-stream table rotations).
#   - s = ||c||^2 is NOT computed by squaring c (a 1024-column ACT pass
#     per tile): with G = A A^T (101x101, host-precomputed, A=[W3S;b3S])
#     s = diag(h2e^T G h2e), two tiny PE matmuls per tile plus a DVE
#     identity-mask reduce.  This keeps the serial ACT queue down to
#     tanh/exp work.
#   - z is transposed, cast to bf16, AND permuted into the exact SBUF
#     tile layout on the host, so every DMA is 128 contiguous line runs
#     (descriptor generation costs ~5ns/line on the issuing sequencer
#     and dominates with scattered layouts).
#   - the batch (1024 samples/core) is processed in EIGHT column groups
#     of 128 samples, with z tiles drawn from a 4-buffer pool: the
#     pool's write-after-read dependency paces group k+4's DMA behind
#     group k's mm1, staggering completions to match consumption
#     (otherwise the SDMA engines round-robin across all queued loads
#     and bunch every completion at the end of the stream).
#
# Sharding: pure data parallelism, batch 8192 -> 8 shards of 1024 rows.

import numpy as np
import ml_dtypes
from contextlib import ExitStack

import concourse.bass as bass
import concourse.tile as tile
from concourse import bacc, mybir
from concourse.bass_utils import run_bass_kernel_spmd

F32 = mybir.dt.float32
F32R = mybir.dt.float32r
BF16 = mybir.dt.bfloat16

N_CORES = 8
BATCH = 8192
B = BATCH // N_CORES      # batch rows per core
D = 3072                  # state dim
H = 100                   # hidden
HP = H + 1                # hidden + ones row (b3 fold)
CD = 1024                 # control dim
NCH = D // 128            # 24 contraction chunks for mm1
NG = 8                    # column groups (= 128-sample output tiles)
GC = 128                  # columns per group
MASS = 0.5

# Newton solve for W(x): w += GAMMA*(x*e^-w - w), seeded by a clipped
# quadratic in x (seed err < 0.05 so the damped step never needs a clamp)
GAMMA = 0.0869
SEED_C0 = 8.73581887
SEED_C1 = 0.70224051e-5
SEED_C2 = -0.06159735e-10

AF = mybir.ActivationFunctionType
ALU = mybir.AluOpType


def build_kernel():
    nc = bacc.Bacc(None, target_bir_lowering=False, enable_partition_id=False)

    zg_d = [
        nc.declare_dram_parameter(f"zg{g}", [128, NCH * GC], BF16, isOutput=False)
        for g in range(NG)
    ]
    tw_d = nc.declare_dram_parameter("tw", [1, B + 128], BF16, isOutput=False)
    w1m_d = nc.declare_dram_parameter("w1m", [128, NCH * 128], BF16, isOutput=False)
    w2b_d = nc.declare_dram_parameter("w2b", [128, 132], F32R, isOutput=False)
    w3s_d = nc.declare_dram_parameter("w3s", [128, CD], BF16, isOutput=False)
    gm_d = nc.declare_dram_parameter("gm", [128, 128], BF16, isOutput=False)
    id_d = nc.declare_dram_parameter("ident", [128, 128], BF16, isOutput=False)
    out_d = nc.declare_dram_parameter("out", [B, CD], BF16, isOutput=True)

    with ExitStack() as ctx:
        tc = ctx.enter_context(tile.TileContext(nc))

        const = ctx.enter_context(tc.tile_pool(name="const", bufs=1))
        zsp = ctx.enter_context(tc.tile_pool(name="zs", bufs=4))
        hsp = ctx.enter_context(tc.tile_pool(name="hs", bufs=2))
        opool = ctx.enter_context(tc.tile_pool(name="outs", bufs=8))
        lwp = ctx.enter_context(tc.tile_pool(name="lw", bufs=1))
        hp_ps = ctx.enter_context(tc.tile_pool(name="hp", bufs=2, space="PSUM"))
        c_ps = ctx.enter_context(tc.tile_pool(name="cp", bufs=3, space="PSUM"))

        # ---- loads, all on the sync HWDGE ring.  The zs pool (bufs=4)
        # stalls z[k+4]'s issue until mm1 has consumed z[k].
        tw = const.tile([1, B + 128], BF16, tag="tw")
        nc.sync.dma_start(tw[:], tw_d[:])

        zg = {}

        def load_z(g):
            zt = zsp.tile([128, NCH, GC], BF16, tag="zs", name=f"zg{g}")
            nc.sync.dma_start(
                zt[:], zg_d[g][:].rearrange("p (c n) -> p c n", c=NCH)
            )
            zg[g] = zt

        w1s = const.tile([128, NCH, 128], BF16, tag="w1s")
        nc.sync.dma_start(w1s[:], w1m_d[:].rearrange("p (c h) -> p c h", c=NCH))
        w2b = const.tile([128, 132], F32R, tag="w2b")
        nc.sync.dma_start(w2b[:], w2b_d[:])
        load_z(0)
        gmat = const.tile([128, 128], BF16, tag="gm")
        nc.sync.dma_start(gmat[:], gm_d[:])
        ident = const.tile([128, 128], BF16, tag="ident")
        nc.sync.dma_start(ident[:], id_d[:])
        load_z(1)
        w3s = const.tile([128, CD], BF16, tag="w3s")
        nc.sync.dma_start(w3s[:], w3s_d[:])
        for g in range(2, NG):
            load_z(g)

        w2 = w2b[0:HP, 0:128]
        b1c = w2b[0:HP, 128:129].bitcast(F32)
        b2c = w2b[0:HP, 129:130].bitcast(F32)
        w1e = tw[0:1, B:B + 128]

        dscr = lwp.tile([128, 128], BF16, tag="dscr")
        x_all = lwp.tile([128, NG], F32, tag="x_all")
        wv = lwp.tile([128, NG], F32, tag="wv")
        kv = lwp.tile([128, NG], F32, tag="kv")
        kvm = lwp.tile([128, NG], F32, tag="kvm")

        h1ps = {}
        h1ss = {}
        h2ss = {}
        cps = {}
        stored = []

        def emit_opener(g):
            h1p = hp_ps.tile([128, GC], F32, tag="hp", name=f"h1p{g}")
            nc.tensor.matmul(
                h1p[:], w1e, tw[0:1, g * GC:(g + 1) * GC],
                start=True, stop=False,
            )
            h1ps[g] = h1p

        def emit_mm1(g):
            h1p = h1ps[g]
            for j in range(NCH):
                nc.tensor.matmul(
                    h1p[:], w1s[:, j, :], zg[g][:, j, :],
                    start=False, stop=(j == NCH - 1),
                )
            del zg[g]

        def emit_tanh1(g):
            h1s = hsp.tile([HP, GC], F32R, tag="h1s", name=f"h1s{g}")
            nc.scalar.activation(
                h1s[:], h1ps.pop(g)[0:HP, :], AF.Tanh, bias=b1c
            )
            h1ss[g] = h1s

        def emit_mm2_tanh2(g):
            h2p = hp_ps.tile([128, GC], F32, tag="hp", name=f"h2p{g}")
            nc.tensor.matmul(
                h2p[:], w2, h1ss.pop(g)[:], start=True, stop=True,
            )
            h2s = hsp.tile([HP, GC], BF16, tag="h2s", name=f"h2s{g}")
            nc.scalar.activation(
                h2s[:], h2p[0:HP, :], AF.Tanh, bias=b2c
            )
            h2ss[g] = h2s

        def emit_gchain(g):
            # s = diag(h2e^T G h2e): P1 = G h2e (PE), P1 -> SBUF (DVE),
            # M = h2e^T P1 (PE), s = rowsum(M * I) (DVE), all tiny ops.
            h2s = h2ss[g]
            p1p = hp_ps.tile([128, GC], F32, tag="hp", name=f"p1{g}")
            nc.tensor.matmul(p1p[:], gmat[0:HP, :], h2s[:], start=True, stop=True)
            p1s = hsp.tile([HP, GC], BF16, tag="p1s", name=f"p1s{g}")
            nc.vector.tensor_copy(p1s[:], p1p[0:HP, :])
            mp = hp_ps.tile([128, GC], F32, tag="hp", name=f"m{g}")
            nc.tensor.matmul(mp[:], h2s[:], p1s[:], start=True, stop=True)
            nc.vector.scalar_tensor_tensor(
                dscr[:], mp[:], 1.0, ident[:], ALU.mult, ALU.mult,
                accum_out=x_all[:, g:g + 1],
            )

        def emit_mm3(g):
            h2s = h2ss.pop(g)
            cp = c_ps.tile([128, CD], F32, tag="cp", name=f"cp{g}")
            for nb in range(2):
                nc.tensor.matmul(
                    cp[:, nb * 512:(nb + 1) * 512],
                    h2s[:],
                    w3s[0:HP, nb * 512:(nb + 1) * 512],
                    start=True, stop=True,
                )
            cps[g] = cp

        def emit_x(sl):
            nc.vector.tensor_scalar(
                x_all[:, sl], x_all[:, sl], 256.0, 8.0, ALU.mult, ALU.add
            )
            n = sl.stop - sl.start
            t = lwp.tile([128, n], F32, tag=f"sd{sl.start}", name="sd")
            nc.vector.tensor_scalar(t[:], x_all[:, sl], SEED_C2, SEED_C1,
                                    ALU.mult, ALU.add)
            nc.vector.tensor_mul(t[:], t[:], x_all[:, sl])
            nc.vector.tensor_scalar(wv[:, sl], t[:], SEED_C0, 8.5,
                                    ALU.add, ALU.max)
            nc.vector.tensor_scalar_min(wv[:, sl], wv[:, sl], 13.0)

        def emit_newton_iter(sl):
            n = sl.stop - sl.start
            em = lwp.tile([128, n], F32, tag=f"em{sl.start}", name="em")
            nc.scalar.activation(em[:], wv[:, sl], AF.Exp, scale=-1.0)
            xem = lwp.tile([128, n], F32, tag=f"xe{sl.start}", name="xe")
            nc.vector.tensor_mul(xem[:], x_all[:, sl], em[:])
            nc.vector.tensor_sub(xem[:], xem[:], wv[:, sl])
            # wv += GAMMA * xem, fused: (xem * GAMMA) + wv
            nc.vector.scalar_tensor_tensor(
                wv[:, sl], xem[:], GAMMA, wv[:, sl], ALU.mult, ALU.add,
            )

        def emit_newton(sl, iters):
            emit_x(sl)
            for _ in range(iters):
                emit_newton_iter(sl)
            nc.scalar.activation(kv[:, sl], wv[:, sl], AF.Exp, scale=-0.5)
            nc.vector.tensor_scalar_mul(kvm[:, sl], kv[:, sl], -256.0)

        def emit_scale_store(g, eng):
            # low half on DVE, high half on ACT: same engine time, half
            # the chain latency before the store can fire
            ot = opool.tile([128, CD], BF16, tag="ot", name="ot")
            cp = cps.pop(g)
            if eng == "dve":
                nc.vector.tensor_scalar(
                    ot[:, 0:512], cp[:, 0:512], kv[:, g:g + 1], -256.0,
                    ALU.mult, ALU.mult,
                )
                nc.scalar.activation(
                    ot[:, 512:1024], cp[:, 512:1024], AF.Copy,
                    scale=kvm[:, g:g + 1],
                )
            else:
                nc.scalar.activation(
                    ot[:, 0:512], cp[:, 0:512], AF.Copy,
                    scale=kvm[:, g:g + 1],
                )
                nc.vector.tensor_scalar(
                    ot[:, 512:1024], cp[:, 512:1024], kv[:, g:g + 1], -256.0,
                    ALU.mult, ALU.mult,
                )
            nc.gpsimd.dma_start(out_d[g * 128:(g + 1) * 128, :], ot[:])
            stored.append(g)

        # ================= main schedule =================
        emit_opener(0)
        emit_mm1(0)
        for k in range(NG):
            emit_tanh1(k)
            if k + 1 < NG:
                emit_opener(k + 1)
                emit_mm1(k + 1)
            emit_mm2_tanh2(k)
            emit_gchain(k)
            if k % 2 == 1 and k < NG - 1:
                # tiles k-1,k: their diags just completed (gchain(k));
                # emit the batch now so it streams a full group earlier
                emit_newton(slice(k - 1, k + 1), iters=2)
            if k == NG - 1:
                # last tile: kv before mm3 so the scale fires immediately
                emit_newton(slice(6, 8), iters=1)
            emit_mm3(k)
            if k >= 2 and k % 2 == 0:
                emit_scale_store(k - 2, "act")
                emit_scale_store(k - 1, "dve")
        # ONE Newton batch over all 8 tiles (a per-2-tile trailing cadence
        # is latency-bound cross-engine ping-pong and can't keep up with
        # the 2.4us/group z pace), then a burst of cheap bf16 scales.
        # loop emitted: newton(0:2)@k2, st(0,1)@k3, newton(2:4)@k4,
        # st(2,3)@k5, newton(4:6)@k6, st(4,5)@k7, newton(6:8)@k7-pre-mm3
        # endgame: split each remaining scale across DVE (low) / ACT (high)
        for g in (6, 7):
            ot = opool.tile([128, CD], BF16, tag="ot", name="ot")
            cp = cps.pop(g)
            nc.vector.tensor_scalar(
                ot[:, 0:512], cp[:, 0:512], kv[:, g:g + 1], -256.0,
                ALU.mult, ALU.mult,
            )
            nc.scalar.activation(
                ot[:, 512:1024], cp[:, 512:1024], AF.Copy,
                scale=kvm[:, g:g + 1],
            )
            nc.sync.dma_start(out_d[g * 128:(g + 1) * 128, :], ot[:])
            stored.append(g)
        assert sorted(stored) == list(range(NG))

    nc.compile()
    return nc


def host_prep(z, t, W1, b1, W2, b2, W3, b3):
    """Host-side weight re-layout + per-core shard maps."""
    f = np.float32
    bf = ml_dtypes.bfloat16
    z = np.asarray(z, f)
    t = np.asarray(t, f)
    W1 = np.asarray(W1, f)
    b1 = np.asarray(b1, f)
    W2 = np.asarray(W2, f)
    b2 = np.asarray(b2, f)
    W3 = np.asarray(W3, f)
    b3 = np.asarray(b3, f)

    # mm1 stationary chunks (bf16, padded to 128 cols):
    # w1m[p, j*128 + h] = W1[1 + j*128 + p, h]
    w1m = np.zeros((128, NCH, 128), bf)
    w1m[:, :, :H] = W1[1:, :].reshape(NCH, 128, H).transpose(1, 0, 2).astype(bf)
    w1m = np.ascontiguousarray(w1m.reshape(128, NCH * 128))

    # w2 padded to [128, 132]: bias columns 128 (b1) and 129 (b2); the
    # 20.0 rows make tanh emit the exact 1.0 ones-row used by the b3 fold
    w2b = np.zeros((128, 132), f)
    w2b[:H, :H] = W2
    w2b[:H, 128] = b1
    w2b[H, 128] = 20.0
    w2b[:H, 129] = b2
    w2b[H, 129] = 20.0

    # fold the p -> c map into W3 (and b3); b3S becomes w3s row 100
    W3r = W3.reshape(H, CD // 4, 12)
    W3S = np.empty((H, CD // 4, 4), f)
    W3S[..., 0] = (W3r[..., 6] + W3r[..., 7] + W3r[..., 8]) / MASS
    W3S[..., 1] = W3r[..., 9]
    W3S[..., 2] = W3r[..., 10]
    W3S[..., 3] = W3r[..., 11]
    b3r = b3.reshape(CD // 4, 12)
    b3S = np.empty((CD // 4, 4), f)
    b3S[..., 0] = (b3r[..., 6] + b3r[..., 7] + b3r[..., 8]) / MASS
    b3S[..., 1] = b3r[..., 9]
    b3S[..., 2] = b3r[..., 10]
    b3S[..., 3] = b3r[..., 11]
    w3s = np.zeros((128, CD), bf)
    w3s[:H] = W3S.reshape(H, CD).astype(bf)
    w3s[H] = b3S.reshape(CD).astype(bf)

    # Gram matrix of A = [W3S; b3S] for the PE-side s computation
    A = np.concatenate([W3S.reshape(H, CD), b3S.reshape(1, CD)], axis=0)
    G = A @ A.T  # [101, 101]
    gm = np.zeros((128, 128), bf)
    gm[:HP, :HP] = G.astype(bf)
    ident = np.eye(128, dtype=bf)

    # z: bf16, transposed, and permuted per column group into the SBUF
    # tile layout [partition, chunk, col] (contiguous per partition)
    zb = z.astype(bf)
    tb = t.astype(bf)
    in_maps = []
    for c in range(N_CORES):
        sl = slice(c * B, (c + 1) * B)
        zt = zb[sl].T  # [D, B] view
        m = {
            "w1m": w1m,
            "w2b": w2b,
            "w3s": w3s,
            "gm": gm,
            "ident": ident,
        }
        for g in range(NG):
            blk = zt[:, g * GC:(g + 1) * GC].reshape(NCH, 128, GC)
            m[f"zg{g}"] = np.ascontiguousarray(
                blk.transpose(1, 0, 2).reshape(128, NCH * GC)
            )
        tw = np.zeros((1, B + 128), bf)
        tw[0, :B] = tb[sl, 0]
        tw[0, B:B + H] = W1[0, :].astype(bf)
        m["tw"] = tw
        in_maps.append(m)
    return in_maps


_NC_CACHE = None


def _get_nc():
    global _NC_CACHE
    if _NC_CACHE is None:
        _NC_CACHE = build_kernel()
    return _NC_CACHE


def run(inputs, trace=False):
    """Returns (full_output, BassKernelResults)."""
    nc = _get_nc()
    in_maps = host_prep(**inputs)
    res = run_bass_kernel_spmd(
        nc, in_maps, list(range(N_CORES)), trace=trace,
    )
    out = np.concatenate(
        [np.asarray(r["out"]).astype(np.float32) for r in res.results], axis=0
    )
    return out, res


def kernel(**inputs):
    out, _ = run(inputs)
    return out


# revision 29
# speedup vs baseline: 1.1364x; 1.1364x over previous
# Trainium2 Bass kernel for nn_CVXPolicy_MultiQuadcopter.
#
# Math (per sample):
#   x  = concat([t, z])                      (3073,)
#   h1 = tanh(x @ W1 + b1)                   (100,)
#   h2 = tanh(h1 @ W2 + b2)                  (100,)
#   p  = h2 @ W3 + b3                        (3072,)
#   c  = S(p)   (per-agent sparse linear map)   (1024,)
#   s  = ||c||^2 ; w = W(256*s) ; k = sqrt(256*w/s)
#   u* = -k * c
#
# Key transformations vs a naive port:
#   - c = S(p) is linear, so S is folded into W3/b3 on the host.
#   - b3 is folded into mm3 via a ones-row (tanh(0+20)==1.0 exactly).
#   - k = 256*exp(-w/2), so the Lambert-W solve needs no sqrt/ln: a
#     damped Newton iteration from a quadratic seed (exp/tanh live in
#     one activation table set -> no mid-stream table rotations).
#   - s = ||c||^2 is NOT computed by squaring c (a 1024-column ACT pass
#     per tile): with G = A A^T (101x101, host-precomputed, A=[W3S;b3S])
#     s = diag(h2e^T G h2e), two tiny PE matmuls per tile plus a DVE
#     identity-mask reduce.  This keeps the serial ACT queue down to
#     tanh/exp work.
#   - z is transposed, cast to bf16, AND permuted into the exact SBUF
#     tile layout on the host, so every DMA is 128 contiguous line runs
#     (descriptor generation costs ~5ns/line on the issuing sequencer
#     and dominates with scattered layouts).
#   - the batch (1024 samples/core) is processed in EIGHT column groups
#     of 128 samples, with z tiles drawn from a 4-buffer pool: the
#     pool's write-after-read dependency paces group k+4's DMA behind
#     group k's mm1, staggering completions to match consumption
#     (otherwise the SDMA engines round-robin across all queued loads
#     and bunch every completion at the end of the stream).
#
# Sharding: pure data parallelism, batch 8192 -> 8 shards of 1024 rows.

import numpy as np
import ml_dtypes
from contextlib import ExitStack

import concourse.bass as bass
import concourse.tile as tile
from concourse import bacc, mybir
from concourse.bass_utils import run_bass_kernel_spmd

F32 = mybir.dt.float32
F32R = mybir.dt.float32r
BF16 = mybir.dt.bfloat16

N_CORES = 8
BATCH = 8192
B = BATCH // N_CORES      # batch rows per core
D = 3072                  # state dim
H = 100                   # hidden
HP = H + 1                # hidden + ones row (b3 fold)
CD = 1024                 # control dim
NCH = D // 128            # 24 contraction chunks for mm1
NG = 8                    # column groups (= 128-sample output tiles)
GC = 128                  # columns per group
MASS = 0.5

# Newton solve for W(x): w += GAMMA*(x*e^-w - w), seeded by a clipped
# quadratic in x (seed err < 0.05 so the damped step never needs a clamp)
GAMMA = 0.0869
SEED_C0 = 8.73581887
SEED_C1 = 0.70224051e-5
SEED_C2 = -0.06159735e-10

AF = mybir.ActivationFunctionType
ALU = mybir.AluOpType


def build_kernel():
    nc = bacc.Bacc(None, target_bir_lowering=False, enable_partition_id=False)

    zg_d = [
        nc.declare_dram_parameter(f"zg{g}", [128, NCH * GC], BF16, isOutput=False)
        for g in range(NG)
    ]
    tw_d = nc.declare_dram_parameter("tw", [1, B + 128], BF16, isOutput=False)
    w1m_d = nc.declare_dram_parameter("w1m", [128, NCH * 128], BF16, isOutput=False)
    w2b_d = nc.declare_dram_parameter("w2b", [128, 132], F32R, isOutput=False)
    w3s_d = nc.declare_dram_parameter("w3s", [128, CD], BF16, isOutput=False)
    gm_d = nc.declare_dram_parameter("gm", [128, 128], BF16, isOutput=False)
    id_d = nc.declare_dram_parameter("ident", [128, 128], BF16, isOutput=False)
    out_d = nc.declare_dram_parameter("out", [B, CD], BF16, isOutput=True)

    with ExitStack() as ctx:
        tc = ctx.enter_context(tile.TileContext(nc))

        const = ctx.enter_context(tc.tile_pool(name="const", bufs=1))
        zsp = ctx.enter_context(tc.tile_pool(name="zs", bufs=4))
        hsp = ctx.enter_context(tc.tile_pool(name="hs", bufs=2))
        opool = ctx.enter_context(tc.tile_pool(name="outs", bufs=8))
        lwp = ctx.enter_context(tc.tile_pool(name="lw", bufs=1))
        hp_ps = ctx.enter_context(tc.tile_pool(name="hp", bufs=2, space="PSUM"))
        c_ps = ctx.enter_context(tc.tile_pool(name="cp", bufs=3, space="PSUM"))

        # ---- loads, all on the sync HWDGE ring.  The zs pool (bufs=4)
        # stalls z[k+4]'s issue until mm1 has consumed z[k].
        tw = const.tile([1, B + 128], BF16, tag="tw")
        nc.sync.dma_start(tw[:], tw_d[:])

        zg = {}

        def load_z(g):
            zt = zsp.tile([128, NCH, GC], BF16, tag="zs", name=f"zg{g}")
            nc.sync.dma_start(
                zt[:], zg_d[g][:].rearrange("p (c n) -> p c n", c=NCH)
            )
            zg[g] = zt

        w1s = const.tile([128, NCH, 128], BF16, tag="w1s")
        nc.sync.dma_start(w1s[:], w1m_d[:].rearrange("p (c h) -> p c h", c=NCH))
        w2b = const.tile([128, 132], F32R, tag="w2b")
        nc.sync.dma_start(w2b[:], w2b_d[:])
        load_z(0)
        gmat = const.tile([128, 128], BF16, tag="gm")
        nc.sync.dma_start(gmat[:], gm_d[:])
        ident = const.tile([128, 128], BF16, tag="ident")
        nc.sync.dma_start(ident[:], id_d[:])
        load_z(1)
        w3s = const.tile([128, CD], BF16, tag="w3s")
        nc.sync.dma_start(w3s[:], w3s_d[:])
        for g in range(2, NG):
            load_z(g)

        w2 = w2b[0:HP, 0:128]
        b1c = w2b[0:HP, 128:129].bitcast(F32)
        b2c = w2b[0:HP, 129:130].bitcast(F32)
        w1e = tw[0:1, B:B + 128]

        dscr = lwp.tile([128, 128], BF16, tag="dscr")
        x_all = lwp.tile([128, NG], F32, tag="x_all")
        wv = lwp.tile([128, NG], F32, tag="wv")
        kv = lwp.tile([128, NG], F32, tag="kv")
        kvm = lwp.tile([128, NG], F32, tag="kvm")

        h1ps = {}
        h1ss = {}
        h2ss = {}
        cps = {}
        stored = []

        def emit_opener(g):
            h1p = hp_ps.tile([128, GC], F32, tag="hp", name=f"h1p{g}")
            nc.tensor.matmul(
                h1p[:], w1e, tw[0:1, g * GC:(g + 1) * GC],
                start=True, stop=False,
            )
            h1ps[g] = h1p

        def emit_mm1(g):
            h1p = h1ps[g]
            for j in range(NCH):
                nc.tensor.matmul(
                    h1p[:], w1s[:, j, :], zg[g][:, j, :],
                    start=False, stop=(j == NCH - 1),
                )
            del zg[g]

        def emit_tanh1(g):
            h1s = hsp.tile([HP, GC], F32R, tag="h1s", name=f"h1s{g}")
            nc.scalar.activation(
                h1s[:], h1ps.pop(g)[0:HP, :], AF.Tanh, bias=b1c
            )
            h1ss[g] = h1s

        def emit_mm2_tanh2(g):
            h2p = hp_ps.tile([128, GC], F32, tag="hp", name=f"h2p{g}")
            nc.tensor.matmul(
                h2p[:], w2, h1ss.pop(g)[:], start=True, stop=True,
            )
            h2s = hsp.tile([HP, GC], BF16, tag="h2s", name=f"h2s{g}")
            nc.scalar.activation(
                h2s[:], h2p[0:HP, :], AF.Tanh, bias=b2c
            )
            h2ss[g] = h2s

        def emit_gchain(g):
            # s = diag(h2e^T G h2e): P1 = G h2e (PE), P1 -> SBUF (DVE),
            # M = h2e^T P1 (PE), s = rowsum(M * I) (DVE), all tiny ops.
            h2s = h2ss[g]
            p1p = hp_ps.tile([128, GC], F32, tag="hp", name=f"p1{g}")
            nc.tensor.matmul(p1p[:], gmat[0:HP, :], h2s[:], start=True, stop=True)
            p1s = hsp.tile([HP, GC], BF16, tag="p1s", name=f"p1s{g}")
            nc.vector.tensor_copy(p1s[:], p1p[0:HP, :])
            mp = hp_ps.tile([128, GC], F32, tag="hp", name=f"m{g}")
            nc.tensor.matmul(mp[:], h2s[:], p1s[:], start=True, stop=True)
            nc.vector.scalar_tensor_tensor(
                dscr[:], mp[:], 1.0, ident[:], ALU.mult, ALU.mult,
                accum_out=x_all[:, g:g + 1],
            )

        def emit_mm3(g):
            h2s = h2ss.pop(g)
            cp = c_ps.tile([128, CD], F32, tag="cp", name=f"cp{g}")
            for nb in range(2):
                nc.tensor.matmul(
                    cp[:, nb * 512:(nb + 1) * 512],
                    h2s[:],
                    w3s[0:HP, nb * 512:(nb + 1) * 512],
                    start=True, stop=True,
                )
            cps[g] = cp

        def emit_x(sl):
            nc.vector.tensor_scalar(
                x_all[:, sl], x_all[:, sl], 256.0, 8.0, ALU.mult, ALU.add
            )
            n = sl.stop - sl.start
            t = lwp.tile([128, n], F32, tag=f"sd{sl.start}", name="sd")
            nc.vector.tensor_scalar(t[:], x_all[:, sl], SEED_C2, SEED_C1,
                                    ALU.mult, ALU.add)
            nc.vector.tensor_mul(t[:], t[:], x_all[:, sl])
            nc.vector.tensor_scalar(wv[:, sl], t[:], SEED_C0, 8.5,
                                    ALU.add, ALU.max)
            nc.vector.tensor_scalar_min(wv[:, sl], wv[:, sl], 13.0)

        def emit_newton_iter(sl):
            n = sl.stop - sl.start
            em = lwp.tile([128, n], F32, tag=f"em{sl.start}", name="em")
            nc.scalar.activation(em[:], wv[:, sl], AF.Exp, scale=-1.0)
            xem = lwp.tile([128, n], F32, tag=f"xe{sl.start}", name="xe")
            nc.vector.tensor_mul(xem[:], x_all[:, sl], em[:])
            nc.vector.tensor_sub(xem[:], xem[:], wv[:, sl])
            # wv += GAMMA * xem, fused: (xem * GAMMA) + wv
            nc.vector.scalar_tensor_tensor(
                wv[:, sl], xem[:], GAMMA, wv[:, sl], ALU.mult, ALU.add,
            )

        def emit_newton(sl, iters):
            emit_x(sl)
            for _ in range(iters):
                emit_newton_iter(sl)
            nc.scalar.activation(kv[:, sl], wv[:, sl], AF.Exp, scale=-0.5)
            nc.vector.tensor_scalar_mul(kvm[:, sl], kv[:, sl], -256.0)

        def emit_scale_store(g, eng):
            ot = opool.tile([128, CD], BF16, tag="ot", name="ot")
            if eng == "dve":
                nc.vector.tensor_scalar(
                    ot[:], cps.pop(g)[:], kv[:, g:g + 1], -256.0,
                    ALU.mult, ALU.mult,
                )
            else:
                nc.scalar.activation(
                    ot[:], cps.pop(g)[:], AF.Copy, scale=kvm[:, g:g + 1]
                )
            nc.gpsimd.dma_start(out_d[g * 128:(g + 1) * 128, :], ot[:])
            stored.append(g)

        # ================= main schedule =================
        emit_opener(0)
        emit_mm1(0)
        for k in range(NG):
            emit_tanh1(k)
            if k + 1 < NG:
                emit_opener(k + 1)
                emit_mm1(k + 1)
            emit_mm2_tanh2(k)
            emit_gchain(k)
            if k % 2 == 1 and k < NG - 1:
                # tiles k-1,k: their diags just completed (gchain(k));
                # emit the batch now so it streams a full group earlier
                emit_newton(slice(k - 1, k + 1), iters=2)
            if k == NG - 1:
                # last tile: kv before mm3 so the scale fires immediately
                emit_newton(slice(6, 8), iters=1)
            emit_mm3(k)
            if k >= 2 and k % 2 == 0:
                emit_scale_store(k - 2, "act")
                emit_scale_store(k - 1, "dve")
        # ONE Newton batch over all 8 tiles (a per-2-tile trailing cadence
        # is latency-bound cross-engine ping-pong and can't keep up with
        # the 2.4us/group z pace), then a burst of cheap bf16 scales.
        # loop emitted: newton(0:2)@k2, st(0,1)@k3, newton(2:4)@k4,
        # st(2,3)@k5, newton(4:6)@k6, st(4,5)@k7, newton(6:8)@k7-pre-mm3
        # endgame: split each remaining scale across DVE (low) / ACT (high)
        for g in (6, 7):
            ot = opool.tile([128, CD], BF16, tag="ot", name="ot")
            cp = cps.pop(g)
            nc.vector.tensor_scalar(
                ot[:, 0:512], cp[:, 0:512], kv[:, g:g + 1], -256.0,
                ALU.mult, ALU.mult,
            )
            nc.scalar.activation(
                ot[:, 512:1024], cp[:, 512:1024], AF.Copy,
                scale=kvm[:, g:g + 1],
            )
            nc.sync.dma_start(out_d[g * 128:(g + 1) * 128, :], ot[:])
            stored.append(g)
        assert sorted(stored) == list(range(NG))

    nc.compile()
    return nc


def host_prep(z, t, W1, b1, W2, b2, W3, b3):
    """Host-side weight re-layout + per-core shard maps."""
    f = np.float32
    bf = ml_dtypes.bfloat16
    z = np.asarray(z, f)
    t = np.asarray(t, f)
    W1 = np.asarray(W1, f)
    b1 = np.asarray(b1, f)
    W2 = np.asarray(W2, f)
    b2 = np.asarray(b2, f)
    W3 = np.asarray(W3, f)
    b3 = np.asarray(b3, f)

    # mm1 stationary chunks (bf16, padded to 128 cols):
    # w1m[p, j*128 + h] = W1[1 + j*128 + p, h]
    w1m = np.zeros((128, NCH, 128), bf)
    w1m[:, :, :H] = W1[1:, :].reshape(NCH, 128, H).transpose(1, 0, 2).astype(bf)
    w1m = np.ascontiguousarray(w1m.reshape(128, NCH * 128))

    # w2 padded to [128, 132]: bias columns 128 (b1) and 129 (b2); the
    # 20.0 rows make tanh emit the exact 1.0 ones-row used by the b3 fold
    w2b = np.zeros((128, 132), f)
    w2b[:H, :H] = W2
    w2b[:H, 128] = b1
    w2b[H, 128] = 20.0
    w2b[:H, 129] = b2
    w2b[H, 129] = 20.0

    # fold the p -> c map into W3 (and b3); b3S becomes w3s row 100
    W3r = W3.reshape(H, CD // 4, 12)
    W3S = np.empty((H, CD // 4, 4), f)
    W3S[..., 0] = (W3r[..., 6] + W3r[..., 7] + W3r[..., 8]) / MASS
    W3S[..., 1] = W3r[..., 9]
    W3S[..., 2] = W3r[..., 10]
    W3S[..., 3] = W3r[..., 11]
    b3r = b3.reshape(CD // 4, 12)
    b3S = np.empty((CD // 4, 4), f)
    b3S[..., 0] = (b3r[..., 6] + b3r[..., 7] + b3r[..., 8]) / MASS
    b3S[..., 1] = b3r[..., 9]
    b3S[..., 2] = b3r[..., 10]
    b3S[..., 3] = b3r[..., 11]
    w3s = np.zeros((128, CD), bf)
    w3s[:H] = W3S.reshape(H, CD).astype(bf)
    w3s[H] = b3S.reshape(CD).astype(bf)

    # Gram matrix of A = [W3S; b3S] for the PE-side s computation
    A = np.concatenate([W3S.reshape(H, CD), b3S.reshape(1, CD)], axis=0)
    G = A @ A.T  # [101, 101]
    gm = np.zeros((128, 128), bf)
    gm[:HP, :HP] = G.astype(bf)
    ident = np.eye(128, dtype=bf)

    # z: bf16, transposed, and permuted per column group into the SBUF
    # tile layout [partition, chunk, col] (contiguous per partition)
    zb = z.astype(bf)
    tb = t.astype(bf)
    in_maps = []
    for c in range(N_CORES):
        sl = slice(c * B, (c + 1) * B)
        zt = zb[sl].T  # [D, B] view
        m = {
            "w1m": w1m,
            "w2b": w2b,
            "w3s": w3s,
            "gm": gm,
            "ident": ident,
        }
        for g in range(NG):
            blk = zt[:, g * GC:(g + 1) * GC].reshape(NCH, 128, GC)
            m[f"zg{g}"] = np.ascontiguousarray(
                blk.transpose(1, 0, 2).reshape(128, NCH * GC)
            )
        tw = np.zeros((1, B + 128), bf)
        tw[0, :B] = tb[sl, 0]
        tw[0, B:B + H] = W1[0, :].astype(bf)
        m["tw"] = tw
        in_maps.append(m)
    return in_maps


_NC_CACHE = None


def _get_nc():
    global _NC_CACHE
    if _NC_CACHE is None:
        _NC_CACHE = build_kernel()
    return _NC_CACHE


def run(inputs, trace=False):
    """Returns (full_output, BassKernelResults)."""
    nc = _get_nc()
    in_maps = host_prep(**inputs)
    res = run_bass_kernel_spmd(
        nc, in_maps, list(range(N_CORES)), trace=trace,
    )
    out = np.concatenate(
        [np.asarray(r["out"]).astype(np.float32) for r in res.results], axis=0
    )
    return out, res


def kernel(**inputs):
    out, _ = run(inputs)
    return out


# revision 30
# speedup vs baseline: 1.1413x; 1.0043x over previous
# Trainium2 Bass kernel for nn_CVXPolicy_MultiQuadcopter.
#
# Math (per sample):
#   x  = concat([t, z])                      (3073,)
#   h1 = tanh(x @ W1 + b1)                   (100,)
#   h2 = tanh(h1 @ W2 + b2)                  (100,)
#   p  = h2 @ W3 + b3                        (3072,)
#   c  = S(p)   (per-agent sparse linear map)   (1024,)
#   s  = ||c||^2 ; w = W(256*s) ; k = sqrt(256*w/s)
#   u* = -k * c
#
# Key transformations vs a naive port:
#   - c = S(p) is linear, so S is folded into W3/b3 on the host.
#   - b3 is folded into mm3 via a ones-row (tanh(0+20)==1.0 exactly).
#   - k = 256*exp(-w/2), so the Lambert-W solve needs no sqrt/ln: a
#     damped Newton iteration from a quadratic seed (exp/tanh live in
#     one activation table set -> no mid-stream table rotations).
#   - s = ||c||^2 is NOT computed by squaring c (a 1024-column ACT pass
#     per tile): with G = A A^T (101x101, host-precomputed, A=[W3S;b3S])
#     s = diag(h2e^T G h2e), two tiny PE matmuls per tile plus a DVE
#     identity-mask reduce.  This keeps the serial ACT queue down to
#     tanh/exp work.
#   - z is transposed, cast to bf16, AND permuted into the exact SBUF
#     tile layout on the host, so every DMA is 128 contiguous line runs
#     (descriptor generation costs ~5ns/line on the issuing sequencer
#     and dominates with scattered layouts).
#   - the batch (1024 samples/core) is processed in EIGHT column groups
#     of 128 samples, with z tiles drawn from a 4-buffer pool: the
#     pool's write-after-read dependency paces group k+4's DMA behind
#     group k's mm1, staggering completions to match consumption
#     (otherwise the SDMA engines round-robin across all queued loads
#     and bunch every completion at the end of the stream).
#
# Sharding: pure data parallelism, batch 8192 -> 8 shards of 1024 rows.

import numpy as np
import ml_dtypes
from contextlib import ExitStack

import concourse.bass as bass
import concourse.tile as tile
from concourse import bacc, mybir
from concourse.bass_utils import run_bass_kernel_spmd

F32 = mybir.dt.float32
F32R = mybir.dt.float32r
BF16 = mybir.dt.bfloat16

N_CORES = 8
BATCH = 8192
B = BATCH // N_CORES      # batch rows per core
D = 3072                  # state dim
H = 100                   # hidden
HP = H + 1                # hidden + ones row (b3 fold)
CD = 1024                 # control dim
NCH = D // 128            # 24 contraction chunks for mm1
NG = 8                    # column groups (= 128-sample output tiles)
GC = 128                  # columns per group
MASS = 0.5

# Newton solve for W(x): w += GAMMA*(x*e^-w - w), seeded by a clipped
# quadratic in x (seed err < 0.05 so the damped step never needs a clamp)
GAMMA = 0.0869
SEED_C0 = 8.73581887
SEED_C1 = 0.70224051e-5
SEED_C2 = -0.06159735e-10

AF = mybir.ActivationFunctionType
ALU = mybir.AluOpType


def build_kernel():
    nc = bacc.Bacc(None, target_bir_lowering=False, enable_partition_id=False)

    zg_d = [
        nc.declare_dram_parameter(f"zg{g}", [128, NCH * GC], BF16, isOutput=False)
        for g in range(NG)
    ]
    tw_d = nc.declare_dram_parameter("tw", [1, B + 128], BF16, isOutput=False)
    w1m_d = nc.declare_dram_parameter("w1m", [128, NCH * 128], BF16, isOutput=False)
    w2b_d = nc.declare_dram_parameter("w2b", [128, 132], F32R, isOutput=False)
    w3s_d = nc.declare_dram_parameter("w3s", [128, CD], BF16, isOutput=False)
    gm_d = nc.declare_dram_parameter("gm", [128, 128], BF16, isOutput=False)
    id_d = nc.declare_dram_parameter("ident", [128, 128], BF16, isOutput=False)
    out_d = nc.declare_dram_parameter("out", [B, CD], BF16, isOutput=True)

    with ExitStack() as ctx:
        tc = ctx.enter_context(tile.TileContext(nc))

        const = ctx.enter_context(tc.tile_pool(name="const", bufs=1))
        zsp = ctx.enter_context(tc.tile_pool(name="zs", bufs=4))
        hsp = ctx.enter_context(tc.tile_pool(name="hs", bufs=2))
        opool = ctx.enter_context(tc.tile_pool(name="outs", bufs=8))
        lwp = ctx.enter_context(tc.tile_pool(name="lw", bufs=1))
        hp_ps = ctx.enter_context(tc.tile_pool(name="hp", bufs=2, space="PSUM"))
        c_ps = ctx.enter_context(tc.tile_pool(name="cp", bufs=3, space="PSUM"))

        # ---- loads, all on the sync HWDGE ring.  The zs pool (bufs=4)
        # stalls z[k+4]'s issue until mm1 has consumed z[k].
        tw = const.tile([1, B + 128], BF16, tag="tw")
        nc.sync.dma_start(tw[:], tw_d[:])

        zg = {}

        def load_z(g):
            zt = zsp.tile([128, NCH, GC], BF16, tag="zs", name=f"zg{g}")
            nc.sync.dma_start(
                zt[:], zg_d[g][:].rearrange("p (c n) -> p c n", c=NCH)
            )
            zg[g] = zt

        w1s = const.tile([128, NCH, 128], BF16, tag="w1s")
        nc.sync.dma_start(w1s[:], w1m_d[:].rearrange("p (c h) -> p c h", c=NCH))
        w2b = const.tile([128, 132], F32R, tag="w2b")
        nc.sync.dma_start(w2b[:], w2b_d[:])
        load_z(0)
        gmat = const.tile([128, 128], BF16, tag="gm")
        nc.sync.dma_start(gmat[:], gm_d[:])
        ident = const.tile([128, 128], BF16, tag="ident")
        nc.sync.dma_start(ident[:], id_d[:])
        load_z(1)
        w3s = const.tile([128, CD], BF16, tag="w3s")
        nc.sync.dma_start(w3s[:], w3s_d[:])
        for g in range(2, NG):
            load_z(g)

        w2 = w2b[0:HP, 0:128]
        b1c = w2b[0:HP, 128:129].bitcast(F32)
        b2c = w2b[0:HP, 129:130].bitcast(F32)
        w1e = tw[0:1, B:B + 128]

        dscr = lwp.tile([128, 128], BF16, tag="dscr")
        x_all = lwp.tile([128, NG], F32, tag="x_all")
        wv = lwp.tile([128, NG], F32, tag="wv")
        kv = lwp.tile([128, NG], F32, tag="kv")
        kvm = lwp.tile([128, NG], F32, tag="kvm")

        h1ps = {}
        h1ss = {}
        h2ss = {}
        cps = {}
        stored = []

        def emit_opener(g):
            h1p = hp_ps.tile([128, GC], F32, tag="hp", name=f"h1p{g}")
            nc.tensor.matmul(
                h1p[:], w1e, tw[0:1, g * GC:(g + 1) * GC],
                start=True, stop=False,
            )
            h1ps[g] = h1p

        def emit_mm1(g):
            h1p = h1ps[g]
            for j in range(NCH):
                nc.tensor.matmul(
                    h1p[:], w1s[:, j, :], zg[g][:, j, :],
                    start=False, stop=(j == NCH - 1),
                )
            del zg[g]

        def emit_tanh1(g):
            h1s = hsp.tile([HP, GC], F32R, tag="h1s", name=f"h1s{g}")
            nc.scalar.activation(
                h1s[:], h1ps.pop(g)[0:HP, :], AF.Tanh, bias=b1c
            )
            h1ss[g] = h1s

        def emit_mm2_tanh2(g):
            h2p = hp_ps.tile([128, GC], F32, tag="hp", name=f"h2p{g}")
            nc.tensor.matmul(
                h2p[:], w2, h1ss.pop(g)[:], start=True, stop=True,
            )
            h2s = hsp.tile([HP, GC], BF16, tag="h2s", name=f"h2s{g}")
            nc.scalar.activation(
                h2s[:], h2p[0:HP, :], AF.Tanh, bias=b2c
            )
            h2ss[g] = h2s

        def emit_gchain(g):
            # s = diag(h2e^T G h2e): P1 = G h2e (PE), P1 -> SBUF (DVE),
            # M = h2e^T P1 (PE), s = rowsum(M * I) (DVE), all tiny ops.
            h2s = h2ss[g]
            p1p = hp_ps.tile([128, GC], F32, tag="hp", name=f"p1{g}")
            nc.tensor.matmul(p1p[:], gmat[0:HP, :], h2s[:], start=True, stop=True)
            p1s = hsp.tile([HP, GC], BF16, tag="p1s", name=f"p1s{g}")
            nc.vector.tensor_copy(p1s[:], p1p[0:HP, :])
            mp = hp_ps.tile([128, GC], F32, tag="hp", name=f"m{g}")
            nc.tensor.matmul(mp[:], h2s[:], p1s[:], start=True, stop=True)
            nc.vector.scalar_tensor_tensor(
                dscr[:], mp[:], 1.0, ident[:], ALU.mult, ALU.mult,
                accum_out=x_all[:, g:g + 1],
            )

        def emit_mm3(g):
            h2s = h2ss.pop(g)
            cp = c_ps.tile([128, CD], F32, tag="cp", name=f"cp{g}")
            for nb in range(2):
                nc.tensor.matmul(
                    cp[:, nb * 512:(nb + 1) * 512],
                    h2s[:],
                    w3s[0:HP, nb * 512:(nb + 1) * 512],
                    start=True, stop=True,
                )
            cps[g] = cp

        def emit_x(sl):
            nc.vector.tensor_scalar(
                x_all[:, sl], x_all[:, sl], 256.0, 8.0, ALU.mult, ALU.add
            )
            n = sl.stop - sl.start
            t = lwp.tile([128, n], F32, tag=f"sd{sl.start}", name="sd")
            nc.vector.tensor_scalar(t[:], x_all[:, sl], SEED_C2, SEED_C1,
                                    ALU.mult, ALU.add)
            nc.vector.tensor_mul(t[:], t[:], x_all[:, sl])
            nc.vector.tensor_scalar(wv[:, sl], t[:], SEED_C0, 8.5,
                                    ALU.add, ALU.max)
            nc.vector.tensor_scalar_min(wv[:, sl], wv[:, sl], 13.0)

        def emit_newton_iter(sl):
            n = sl.stop - sl.start
            em = lwp.tile([128, n], F32, tag=f"em{sl.start}", name="em")
            nc.scalar.activation(em[:], wv[:, sl], AF.Exp, scale=-1.0)
            xem = lwp.tile([128, n], F32, tag=f"xe{sl.start}", name="xe")
            nc.vector.tensor_mul(xem[:], x_all[:, sl], em[:])
            nc.vector.tensor_sub(xem[:], xem[:], wv[:, sl])
            # wv += GAMMA * xem, fused: (xem * GAMMA) + wv
            nc.vector.scalar_tensor_tensor(
                wv[:, sl], xem[:], GAMMA, wv[:, sl], ALU.mult, ALU.add,
            )

        def emit_newton(sl, iters):
            emit_x(sl)
            for _ in range(iters):
                emit_newton_iter(sl)
            nc.scalar.activation(kv[:, sl], wv[:, sl], AF.Exp, scale=-0.5)
            nc.vector.tensor_scalar_mul(kvm[:, sl], kv[:, sl], -256.0)

        def emit_scale_store(g, eng):
            ot = opool.tile([128, CD], BF16, tag="ot", name="ot")
            if eng == "dve":
                nc.vector.tensor_scalar(
                    ot[:], cps.pop(g)[:], kv[:, g:g + 1], -256.0,
                    ALU.mult, ALU.mult,
                )
            else:
                nc.scalar.activation(
                    ot[:], cps.pop(g)[:], AF.Copy, scale=kvm[:, g:g + 1]
                )
            # sync ring: in-order behind every z load, so stores can never
            # preempt the paced z stream mid-loop (a store sneaking into the
            # stream stretches the pacing feedback loop by ~9us on bad runs)
            nc.sync.dma_start(out_d[g * 128:(g + 1) * 128, :], ot[:])
            stored.append(g)

        # ================= main schedule =================
        emit_opener(0)
        emit_mm1(0)
        for k in range(NG):
            emit_tanh1(k)
            if k + 1 < NG:
                emit_opener(k + 1)
                emit_mm1(k + 1)
            emit_mm2_tanh2(k)
            emit_gchain(k)
            if k % 2 == 1 and k < NG - 1:
                # tiles k-1,k: their diags just completed (gchain(k));
                # emit the batch now so it streams a full group earlier
                emit_newton(slice(k - 1, k + 1), iters=2)
            if k == NG - 1:
                # last tile: kv before mm3 so the scale fires immediately
                emit_newton(slice(6, 8), iters=1)
            emit_mm3(k)
            if k >= 2 and k % 2 == 0:
                emit_scale_store(k - 2, "act")
                emit_scale_store(k - 1, "dve")
        # ONE Newton batch over all 8 tiles (a per-2-tile trailing cadence
        # is latency-bound cross-engine ping-pong and can't keep up with
        # the 2.4us/group z pace), then a burst of cheap bf16 scales.
        # loop emitted: newton(0:2)@k2, st(0,1)@k3, newton(2:4)@k4,
        # st(2,3)@k5, newton(4:6)@k6, st(4,5)@k7, newton(6:8)@k7-pre-mm3
        # endgame: split each remaining scale across DVE (low) / ACT (high)
        for g in (6, 7):
            ot = opool.tile([128, CD], BF16, tag="ot", name="ot")
            cp = cps.pop(g)
            nc.vector.tensor_scalar(
                ot[:, 0:512], cp[:, 0:512], kv[:, g:g + 1], -256.0,
                ALU.mult, ALU.mult,
            )
            nc.scalar.activation(
                ot[:, 512:1024], cp[:, 512:1024], AF.Copy,
                scale=kvm[:, g:g + 1],
            )
            nc.sync.dma_start(out_d[g * 128:(g + 1) * 128, :], ot[:])
            stored.append(g)
        assert sorted(stored) == list(range(NG))

    nc.compile()
    return nc


def host_prep(z, t, W1, b1, W2, b2, W3, b3):
    """Host-side weight re-layout + per-core shard maps."""
    f = np.float32
    bf = ml_dtypes.bfloat16
    z = np.asarray(z, f)
    t = np.asarray(t, f)
    W1 = np.asarray(W1, f)
    b1 = np.asarray(b1, f)
    W2 = np.asarray(W2, f)
    b2 = np.asarray(b2, f)
    W3 = np.asarray(W3, f)
    b3 = np.asarray(b3, f)

    # mm1 stationary chunks (bf16, padded to 128 cols):
    # w1m[p, j*128 + h] = W1[1 + j*128 + p, h]
    w1m = np.zeros((128, NCH, 128), bf)
    w1m[:, :, :H] = W1[1:, :].reshape(NCH, 128, H).transpose(1, 0, 2).astype(bf)
    w1m = np.ascontiguousarray(w1m.reshape(128, NCH * 128))

    # w2 padded to [128, 132]: bias columns 128 (b1) and 129 (b2); the
    # 20.0 rows make tanh emit the exact 1.0 ones-row used by the b3 fold
    w2b = np.zeros((128, 132), f)
    w2b[:H, :H] = W2
    w2b[:H, 128] = b1
    w2b[H, 128] = 20.0
    w2b[:H, 129] = b2
    w2b[H, 129] = 20.0

    # fold the p -> c map into W3 (and b3); b3S becomes w3s row 100
    W3r = W3.reshape(H, CD // 4, 12)
    W3S = np.empty((H, CD // 4, 4), f)
    W3S[..., 0] = (W3r[..., 6] + W3r[..., 7] + W3r[..., 8]) / MASS
    W3S[..., 1] = W3r[..., 9]
    W3S[..., 2] = W3r[..., 10]
    W3S[..., 3] = W3r[..., 11]
    b3r = b3.reshape(CD // 4, 12)
    b3S = np.empty((CD // 4, 4), f)
    b3S[..., 0] = (b3r[..., 6] + b3r[..., 7] + b3r[..., 8]) / MASS
    b3S[..., 1] = b3r[..., 9]
    b3S[..., 2] = b3r[..., 10]
    b3S[..., 3] = b3r[..., 11]
    w3s = np.zeros((128, CD), bf)
    w3s[:H] = W3S.reshape(H, CD).astype(bf)
    w3s[H] = b3S.reshape(CD).astype(bf)

    # Gram matrix of A = [W3S; b3S] for the PE-side s computation
    A = np.concatenate([W3S.reshape(H, CD), b3S.reshape(1, CD)], axis=0)
    G = A @ A.T  # [101, 101]
    gm = np.zeros((128, 128), bf)
    gm[:HP, :HP] = G.astype(bf)
    ident = np.eye(128, dtype=bf)

    # z: bf16, transposed, and permuted per column group into the SBUF
    # tile layout [partition, chunk, col] (contiguous per partition)
    zb = z.astype(bf)
    tb = t.astype(bf)
    in_maps = []
    for c in range(N_CORES):
        sl = slice(c * B, (c + 1) * B)
        zt = zb[sl].T  # [D, B] view
        m = {
            "w1m": w1m,
            "w2b": w2b,
            "w3s": w3s,
            "gm": gm,
            "ident": ident,
        }
        for g in range(NG):
            blk = zt[:, g * GC:(g + 1) * GC].reshape(NCH, 128, GC)
            m[f"zg{g}"] = np.ascontiguousarray(
                blk.transpose(1, 0, 2).reshape(128, NCH * GC)
            )
        tw = np.zeros((1, B + 128), bf)
        tw[0, :B] = tb[sl, 0]
        tw[0, B:B + H] = W1[0, :].astype(bf)
        m["tw"] = tw
        in_maps.append(m)
    return in_maps


_NC_CACHE = None


def _get_nc():
    global _NC_CACHE
    if _NC_CACHE is None:
        _NC_CACHE = build_kernel()
    return _NC_CACHE


def run(inputs, trace=False):
    """Returns (full_output, BassKernelResults)."""
    nc = _get_nc()
    in_maps = host_prep(**inputs)
    res = run_bass_kernel_spmd(
        nc, in_maps, list(range(N_CORES)), trace=trace,
    )
    out = np.concatenate(
        [np.asarray(r["out"]).astype(np.float32) for r in res.results], axis=0
    )
    return out, res


def kernel(**inputs):
    out, _ = run(inputs)
    return out


# revision 31
# speedup vs baseline: 1.1460x; 1.0042x over previous
# Trainium2 Bass kernel for nn_CVXPolicy_MultiQuadcopter.
#
# Math (per sample):
#   x  = concat([t, z])                      (3073,)
#   h1 = tanh(x @ W1 + b1)                   (100,)
#   h2 = tanh(h1 @ W2 + b2)                  (100,)
#   p  = h2 @ W3 + b3                        (3072,)
#   c  = S(p)   (per-agent sparse linear map)   (1024,)
#   s  = ||c||^2 ; w = W(256*s) ; k = sqrt(256*w/s)
#   u* = -k * c
#
# Key transformations vs a naive port:
#   - c = S(p) is linear, so S is folded into W3/b3 on the host.
#   - b3 is folded into mm3 via a ones-row (tanh(0+20)==1.0 exactly).
#   - k = 256*exp(-w/2), so the Lambert-W solve needs no sqrt/ln: a
#     damped Newton iteration from a quadratic seed (exp/tanh live in
#     one activation table set -> no mid-stream table rotations).
#   - s = ||c||^2 is NOT computed by squaring c (a 1024-column ACT pass
#     per tile): with G = A A^T (101x101, host-precomputed, A=[W3S;b3S])
#     s = diag(h2e^T G h2e), two tiny PE matmuls per tile plus a DVE
#     identity-mask reduce.  This keeps the serial ACT queue down to
#     tanh/exp work.
#   - z is transposed, cast to bf16, AND permuted into the exact SBUF
#     tile layout on the host, so every DMA is 128 contiguous line runs
#     (descriptor generation costs ~5ns/line on the issuing sequencer
#     and dominates with scattered layouts).
#   - the batch (1024 samples/core) is processed in EIGHT column groups
#     of 128 samples, with z tiles drawn from a 4-buffer pool: the
#     pool's write-after-read dependency paces group k+4's DMA behind
#     group k's mm1, staggering completions to match consumption
#     (otherwise the SDMA engines round-robin across all queued loads
#     and bunch every completion at the end of the stream).
#
# Sharding: pure data parallelism, batch 8192 -> 8 shards of 1024 rows.

import numpy as np
import ml_dtypes
from contextlib import ExitStack

import concourse.bass as bass
import concourse.tile as tile
from concourse import bacc, mybir
from concourse.bass_utils import run_bass_kernel_spmd

F32 = mybir.dt.float32
F32R = mybir.dt.float32r
BF16 = mybir.dt.bfloat16

N_CORES = 8
BATCH = 8192
B = BATCH // N_CORES      # batch rows per core
D = 3072                  # state dim
H = 100                   # hidden
HP = H + 1                # hidden + ones row (b3 fold)
CD = 1024                 # control dim
NCH = D // 128            # 24 contraction chunks for mm1
NG = 8                    # column groups (= 128-sample output tiles)
GC = 128                  # columns per group
MASS = 0.5

# Newton solve for W(x): w += GAMMA*(x*e^-w - w), seeded by a clipped
# quadratic in x (seed err < 0.05 so the damped step never needs a clamp)
GAMMA = 0.0869
SEED_C0 = 8.73581887
SEED_C1 = 0.70224051e-5
SEED_C2 = -0.06159735e-10

AF = mybir.ActivationFunctionType
ALU = mybir.AluOpType


def build_kernel():
    nc = bacc.Bacc(None, target_bir_lowering=False, enable_partition_id=False)

    zg_d = [
        nc.declare_dram_parameter(f"zg{g}", [128, NCH * GC], BF16, isOutput=False)
        for g in range(NG)
    ]
    tw_d = nc.declare_dram_parameter("tw", [1, B + 128], BF16, isOutput=False)
    w1m_d = nc.declare_dram_parameter("w1m", [128, NCH * 128], BF16, isOutput=False)
    w2b_d = nc.declare_dram_parameter("w2b", [128, 132], F32R, isOutput=False)
    w3s_d = nc.declare_dram_parameter("w3s", [128, CD], BF16, isOutput=False)
    gm_d = nc.declare_dram_parameter("gm", [128, 128], BF16, isOutput=False)
    id_d = nc.declare_dram_parameter("ident", [128, 128], BF16, isOutput=False)
    out_d = nc.declare_dram_parameter("out", [B, CD], BF16, isOutput=True)

    with ExitStack() as ctx:
        tc = ctx.enter_context(tile.TileContext(nc))

        const = ctx.enter_context(tc.tile_pool(name="const", bufs=1))
        zsp = ctx.enter_context(tc.tile_pool(name="zs", bufs=4))
        hsp = ctx.enter_context(tc.tile_pool(name="hs", bufs=2))
        opool = ctx.enter_context(tc.tile_pool(name="outs", bufs=8))
        lwp = ctx.enter_context(tc.tile_pool(name="lw", bufs=1))
        hp_ps = ctx.enter_context(tc.tile_pool(name="hp", bufs=2, space="PSUM"))
        c_ps = ctx.enter_context(tc.tile_pool(name="cp", bufs=3, space="PSUM"))

        # ---- loads, all on the sync HWDGE ring.  The zs pool (bufs=4)
        # stalls z[k+4]'s issue until mm1 has consumed z[k].
        tw = const.tile([1, B + 128], BF16, tag="tw")
        nc.sync.dma_start(tw[:], tw_d[:])

        zg = {}

        def load_z(g, parts=1):
            # parts>1: split the transfer so mm1 can start on partial data
            # (range-based dep tracking gates each chunk's matmul on the
            # sub-DMA that covers it) — used for the last groups, whose
            # mm1 otherwise idles a full transfer time after z lands
            zt = zsp.tile([128, NCH, GC], BF16, tag="zs", name=f"zg{g}")
            step = NCH // parts
            for c0 in range(0, NCH, step):
                nc.sync.dma_start(
                    zt[:, c0:c0 + step, :],
                    zg_d[g][:, c0 * GC:(c0 + step) * GC].rearrange(
                        "p (c n) -> p c n", c=step
                    ),
                )
            zg[g] = zt

        w1s = const.tile([128, NCH, 128], BF16, tag="w1s")
        nc.sync.dma_start(w1s[:], w1m_d[:].rearrange("p (c h) -> p c h", c=NCH))
        w2b = const.tile([128, 132], F32R, tag="w2b")
        nc.sync.dma_start(w2b[:], w2b_d[:])
        load_z(0)
        gmat = const.tile([128, 128], BF16, tag="gm")
        nc.sync.dma_start(gmat[:], gm_d[:])
        ident = const.tile([128, 128], BF16, tag="ident")
        nc.sync.dma_start(ident[:], id_d[:])
        load_z(1)
        w3s = const.tile([128, CD], BF16, tag="w3s")
        nc.sync.dma_start(w3s[:], w3s_d[:])
        for g in range(2, NG):
            load_z(g, parts=3 if g >= NG - 2 else 1)

        w2 = w2b[0:HP, 0:128]
        b1c = w2b[0:HP, 128:129].bitcast(F32)
        b2c = w2b[0:HP, 129:130].bitcast(F32)
        w1e = tw[0:1, B:B + 128]

        dscr = lwp.tile([128, 128], BF16, tag="dscr")
        x_all = lwp.tile([128, NG], F32, tag="x_all")
        wv = lwp.tile([128, NG], F32, tag="wv")
        kv = lwp.tile([128, NG], F32, tag="kv")
        kvm = lwp.tile([128, NG], F32, tag="kvm")

        h1ps = {}
        h1ss = {}
        h2ss = {}
        cps = {}
        stored = []

        def emit_opener(g):
            h1p = hp_ps.tile([128, GC], F32, tag="hp", name=f"h1p{g}")
            nc.tensor.matmul(
                h1p[:], w1e, tw[0:1, g * GC:(g + 1) * GC],
                start=True, stop=False,
            )
            h1ps[g] = h1p

        def emit_mm1(g):
            h1p = h1ps[g]
            for j in range(NCH):
                nc.tensor.matmul(
                    h1p[:], w1s[:, j, :], zg[g][:, j, :],
                    start=False, stop=(j == NCH - 1),
                )
            del zg[g]

        def emit_tanh1(g):
            h1s = hsp.tile([HP, GC], F32R, tag="h1s", name=f"h1s{g}")
            nc.scalar.activation(
                h1s[:], h1ps.pop(g)[0:HP, :], AF.Tanh, bias=b1c
            )
            h1ss[g] = h1s

        def emit_mm2_tanh2(g):
            h2p = hp_ps.tile([128, GC], F32, tag="hp", name=f"h2p{g}")
            nc.tensor.matmul(
                h2p[:], w2, h1ss.pop(g)[:], start=True, stop=True,
            )
            h2s = hsp.tile([HP, GC], BF16, tag="h2s", name=f"h2s{g}")
            nc.scalar.activation(
                h2s[:], h2p[0:HP, :], AF.Tanh, bias=b2c
            )
            h2ss[g] = h2s

        def emit_gchain(g):
            # s = diag(h2e^T G h2e): P1 = G h2e (PE), P1 -> SBUF (DVE),
            # M = h2e^T P1 (PE), s = rowsum(M * I) (DVE), all tiny ops.
            h2s = h2ss[g]
            p1p = hp_ps.tile([128, GC], F32, tag="hp", name=f"p1{g}")
            nc.tensor.matmul(p1p[:], gmat[0:HP, :], h2s[:], start=True, stop=True)
            p1s = hsp.tile([HP, GC], BF16, tag="p1s", name=f"p1s{g}")
            nc.vector.tensor_copy(p1s[:], p1p[0:HP, :])
            mp = hp_ps.tile([128, GC], F32, tag="hp", name=f"m{g}")
            nc.tensor.matmul(mp[:], h2s[:], p1s[:], start=True, stop=True)
            nc.vector.scalar_tensor_tensor(
                dscr[:], mp[:], 1.0, ident[:], ALU.mult, ALU.mult,
                accum_out=x_all[:, g:g + 1],
            )

        def emit_mm3(g):
            h2s = h2ss.pop(g)
            cp = c_ps.tile([128, CD], F32, tag="cp", name=f"cp{g}")
            for nb in range(2):
                nc.tensor.matmul(
                    cp[:, nb * 512:(nb + 1) * 512],
                    h2s[:],
                    w3s[0:HP, nb * 512:(nb + 1) * 512],
                    start=True, stop=True,
                )
            cps[g] = cp

        def emit_x(sl):
            nc.vector.tensor_scalar(
                x_all[:, sl], x_all[:, sl], 256.0, 8.0, ALU.mult, ALU.add
            )
            n = sl.stop - sl.start
            t = lwp.tile([128, n], F32, tag=f"sd{sl.start}", name="sd")
            nc.vector.tensor_scalar(t[:], x_all[:, sl], SEED_C2, SEED_C1,
                                    ALU.mult, ALU.add)
            nc.vector.tensor_mul(t[:], t[:], x_all[:, sl])
            nc.vector.tensor_scalar(wv[:, sl], t[:], SEED_C0, 8.5,
                                    ALU.add, ALU.max)
            nc.vector.tensor_scalar_min(wv[:, sl], wv[:, sl], 13.0)

        def emit_newton_iter(sl):
            n = sl.stop - sl.start
            em = lwp.tile([128, n], F32, tag=f"em{sl.start}", name="em")
            nc.scalar.activation(em[:], wv[:, sl], AF.Exp, scale=-1.0)
            xem = lwp.tile([128, n], F32, tag=f"xe{sl.start}", name="xe")
            nc.vector.tensor_mul(xem[:], x_all[:, sl], em[:])
            nc.vector.tensor_sub(xem[:], xem[:], wv[:, sl])
            # wv += GAMMA * xem, fused: (xem * GAMMA) + wv
            nc.vector.scalar_tensor_tensor(
                wv[:, sl], xem[:], GAMMA, wv[:, sl], ALU.mult, ALU.add,
            )

        def emit_newton(sl, iters):
            emit_x(sl)
            for _ in range(iters):
                emit_newton_iter(sl)
            nc.scalar.activation(kv[:, sl], wv[:, sl], AF.Exp, scale=-0.5)
            nc.vector.tensor_scalar_mul(kvm[:, sl], kv[:, sl], -256.0)

        def emit_scale_store(g, eng):
            ot = opool.tile([128, CD], BF16, tag="ot", name="ot")
            if eng == "dve":
                nc.vector.tensor_scalar(
                    ot[:], cps.pop(g)[:], kv[:, g:g + 1], -256.0,
                    ALU.mult, ALU.mult,
                )
            else:
                nc.scalar.activation(
                    ot[:], cps.pop(g)[:], AF.Copy, scale=kvm[:, g:g + 1]
                )
            # sync ring: in-order behind every z load, so stores can never
            # preempt the paced z stream mid-loop (a store sneaking into the
            # stream stretches the pacing feedback loop by ~9us on bad runs)
            nc.sync.dma_start(out_d[g * 128:(g + 1) * 128, :], ot[:])
            stored.append(g)

        # ================= main schedule =================
        emit_opener(0)
        emit_mm1(0)
        for k in range(NG):
            emit_tanh1(k)
            if k + 1 < NG:
                emit_opener(k + 1)
                emit_mm1(k + 1)
            emit_mm2_tanh2(k)
            emit_gchain(k)
            if k % 2 == 1 and k < NG - 1:
                # tiles k-1,k: their diags just completed (gchain(k));
                # emit the batch now so it streams a full group earlier
                emit_newton(slice(k - 1, k + 1), iters=2)
            if k == NG - 1:
                # last tile: kv before mm3 so the scale fires immediately
                emit_newton(slice(6, 8), iters=1)
            emit_mm3(k)
            if k >= 2 and k % 2 == 0:
                emit_scale_store(k - 2, "act")
                emit_scale_store(k - 1, "dve")
        # ONE Newton batch over all 8 tiles (a per-2-tile trailing cadence
        # is latency-bound cross-engine ping-pong and can't keep up with
        # the 2.4us/group z pace), then a burst of cheap bf16 scales.
        # loop emitted: newton(0:2)@k2, st(0,1)@k3, newton(2:4)@k4,
        # st(2,3)@k5, newton(4:6)@k6, st(4,5)@k7, newton(6:8)@k7-pre-mm3
        # endgame: split each remaining scale across DVE (low) / ACT (high)
        for g in (6, 7):
            ot = opool.tile([128, CD], BF16, tag="ot", name="ot")
            cp = cps.pop(g)
            nc.vector.tensor_scalar(
                ot[:, 0:512], cp[:, 0:512], kv[:, g:g + 1], -256.0,
                ALU.mult, ALU.mult,
            )
            nc.scalar.activation(
                ot[:, 512:1024], cp[:, 512:1024], AF.Copy,
                scale=kvm[:, g:g + 1],
            )
            nc.sync.dma_start(out_d[g * 128:(g + 1) * 128, :], ot[:])
            stored.append(g)
        assert sorted(stored) == list(range(NG))

    nc.compile()
    return nc


def host_prep(z, t, W1, b1, W2, b2, W3, b3):
    """Host-side weight re-layout + per-core shard maps."""
    f = np.float32
    bf = ml_dtypes.bfloat16
    z = np.asarray(z, f)
    t = np.asarray(t, f)
    W1 = np.asarray(W1, f)
    b1 = np.asarray(b1, f)
    W2 = np.asarray(W2, f)
    b2 = np.asarray(b2, f)
    W3 = np.asarray(W3, f)
    b3 = np.asarray(b3, f)

    # mm1 stationary chunks (bf16, padded to 128 cols):
    # w1m[p, j*128 + h] = W1[1 + j*128 + p, h]
    w1m = np.zeros((128, NCH, 128), bf)
    w1m[:, :, :H] = W1[1:, :].reshape(NCH, 128, H).transpose(1, 0, 2).astype(bf)
    w1m = np.ascontiguousarray(w1m.reshape(128, NCH * 128))

    # w2 padded to [128, 132]: bias columns 128 (b1) and 129 (b2); the
    # 20.0 rows make tanh emit the exact 1.0 ones-row used by the b3 fold
    w2b = np.zeros((128, 132), f)
    w2b[:H, :H] = W2
    w2b[:H, 128] = b1
    w2b[H, 128] = 20.0
    w2b[:H, 129] = b2
    w2b[H, 129] = 20.0

    # fold the p -> c map into W3 (and b3); b3S becomes w3s row 100
    W3r = W3.reshape(H, CD // 4, 12)
    W3S = np.empty((H, CD // 4, 4), f)
    W3S[..., 0] = (W3r[..., 6] + W3r[..., 7] + W3r[..., 8]) / MASS
    W3S[..., 1] = W3r[..., 9]
    W3S[..., 2] = W3r[..., 10]
    W3S[..., 3] = W3r[..., 11]
    b3r = b3.reshape(CD // 4, 12)
    b3S = np.empty((CD // 4, 4), f)
    b3S[..., 0] = (b3r[..., 6] + b3r[..., 7] + b3r[..., 8]) / MASS
    b3S[..., 1] = b3r[..., 9]
    b3S[..., 2] = b3r[..., 10]
    b3S[..., 3] = b3r[..., 11]
    w3s = np.zeros((128, CD), bf)
    w3s[:H] = W3S.reshape(H, CD).astype(bf)
    w3s[H] = b3S.reshape(CD).astype(bf)

    # Gram matrix of A = [W3S; b3S] for the PE-side s computation
    A = np.concatenate([W3S.reshape(H, CD), b3S.reshape(1, CD)], axis=0)
    G = A @ A.T  # [101, 101]
    gm = np.zeros((128, 128), bf)
    gm[:HP, :HP] = G.astype(bf)
    ident = np.eye(128, dtype=bf)

    # z: bf16, transposed, and permuted per column group into the SBUF
    # tile layout [partition, chunk, col] (contiguous per partition)
    zb = z.astype(bf)
    tb = t.astype(bf)
    in_maps = []
    for c in range(N_CORES):
        sl = slice(c * B, (c + 1) * B)
        zt = zb[sl].T  # [D, B] view
        m = {
            "w1m": w1m,
            "w2b": w2b,
            "w3s": w3s,
            "gm": gm,
            "ident": ident,
        }
        for g in range(NG):
            blk = zt[:, g * GC:(g + 1) * GC].reshape(NCH, 128, GC)
            m[f"zg{g}"] = np.ascontiguousarray(
                blk.transpose(1, 0, 2).reshape(128, NCH * GC)
            )
        tw = np.zeros((1, B + 128), bf)
        tw[0, :B] = tb[sl, 0]
        tw[0, B:B + H] = W1[0, :].astype(bf)
        m["tw"] = tw
        in_maps.append(m)
    return in_maps


_NC_CACHE = None


def _get_nc():
    global _NC_CACHE
    if _NC_CACHE is None:
        _NC_CACHE = build_kernel()
    return _NC_CACHE


def run(inputs, trace=False):
    """Returns (full_output, BassKernelResults)."""
    nc = _get_nc()
    in_maps = host_prep(**inputs)
    res = run_bass_kernel_spmd(
        nc, in_maps, list(range(N_CORES)), trace=trace,
    )
    out = np.concatenate(
        [np.asarray(r["out"]).astype(np.float32) for r in res.results], axis=0
    )
    return out, res


def kernel(**inputs):
    out, _ = run(inputs)
    return out
